# revision 26
# baseline (speedup 1.0000x reference)
"""Deformable conv (dense_cnn) Trainium2 kernel, SPMD over 8 NeuronCores.

Strategy (v2)
-------------
Sharding: 8 cores = 4 images x 2 vertical strips of 64 output columns.

Math: torchvision deform_conv2d semantics via "tri-window masked shift":

    sampled_k[h, w, c] = sum_{my,nx} tri(dy_k - my) * tri(dx_k - nx)
                         * x[h + ky-1+my, w + kx-1+nx, c]

with tri(t) = relu(1 - |t|), exact for |d| <= 1 (zero-padded x).

v2 layout choices (all driven by the DVE perf-mode rules):
  * modulate runs in [h-part, (c, w)] layout with w INNERMOST so every
    tensor_tensor has a stride-1 innermost dim on all operands -> 2x_1P
    DVE mode (the old [., (w, c)] layout put the coefficient broadcast
    on the innermost dim -> stride 0 -> 1x mode).
  * x is staged as 5 row-shift planes x 2 column-parity copies per
    superchunk (streamed from DRAM, double-buffered) so all shifted
    reads stay 4-byte aligned (another 2x_1P requirement).
  * the 3 my-shifts of a tap share one fused mult op (regular stride
    across shift planes); 3 fused mults + 8 adds per (tap, superchunk).
    One fused mult per tap runs on GPSIMD, the rest on DVE.
  * transposes accumulate per-tap via k-outer loop; the contraction
    PSUM tile [o, (w,h)] accumulates over taps, so no big staging
    buffer; PSUM->SBUF copies run on the otherwise-idle ScalarE.
  * tri coefficients are computed on ScalarE (Abs/Relu activations with
    fused scale/bias).
"""

import numpy as np
import ml_dtypes

B, C, H, W, O = 4, 128, 128, 128, 128
K2 = 9
SW = 64          # strip width (output columns per core)
NCORES = 8
SC_W = 16        # superchunk width (output columns per inner block)
NSC = SW // SC_W
NS = 5           # row shifts -2..2
NQ = 2           # column parity copies
WQ = 20          # w slots per (sc, s, q) plane: w0-2 .. w0+17
SC_ELEMS = NS * NQ * C * WQ   # xq elements per (partition, superchunk)

f16 = np.float16

# set False to run all modulate ops on DVE (debug/perf toggle)
# (measured: GPSIMD shares an SBUF port with the DVE; a concurrent GPSIMD
# tensor_tensor stretches the overlapping DVE op ~10x, a large net loss)
USE_GPSIMD = False
# reduce the ni=2 product planes with SDMA CCE accumulate-DMAs instead of
# DVE adds (SDMA engines are otherwise idle)
USE_DMA_ACCUM = True


def _build_nc():
    import concourse.bacc as bacc
    import concourse.mybir as mybir
    from concourse.tile import TileContext
    from concourse.masks import make_identity

    nc = bacc.Bacc()
    dt = mybir.dt

    # ---- DRAM params (per-core shards, host-prepared layouts) ----
    # xq: [h=128, sc(4) x s(5) x q(2) x c(128) x w(20)] bf16
    #   xq[h, sc, s, q, c, w'] = x[c, h+s-2, ws + 16*sc - 2 + q + w'] (0 pad)
    xq_d = nc.declare_dram_parameter("xq", [128, NSC * SC_ELEMS], dt.float16, isOutput=False)
    # xcm: [c=128, 130 h x 66 w] bf16 (rows -1..128, cols ws-1..ws+65, zero pad)
    xcm_d = nc.declare_dram_parameter("xcm", [128, 130 * 66], dt.float16, isOutput=False)
    # woff: [c=128, 9 k x 18 t] bf16   (rhs tiles, per tap)
    woff_d = nc.declare_dram_parameter("woff", [128, K2 * 18], dt.float16, isOutput=False)
    # wconv: [c=128, 9 k x 128 o] bf16  (lhsT tiles, per tap)
    wconv_d = nc.declare_dram_parameter("wconv", [128, K2 * 128], dt.float16, isOutput=False)
    # out: [o=128, 64 w x 128 h] bf16 (j = w*128 + h ordering)
    out_d = nc.declare_dram_parameter("out", [128, SW * 128], dt.float16, isOutput=True)

    with TileContext(nc) as tc:
        with (
            tc.tile_pool(name="const", bufs=1) as constp,
            tc.tile_pool(name="xqp", bufs=2) as xqp,
            tc.tile_pool(name="xcmp", bufs=1) as xcmp,
            tc.tile_pool(name="offp", bufs=1) as offp,
            tc.tile_pool(name="coefp", bufs=1) as coefp,
            tc.tile_pool(name="prodp", bufs=4) as prodp,
            tc.tile_pool(name="sampp", bufs=3) as sampp,
            tc.tile_pool(name="stagep", bufs=1) as stagep,
            tc.tile_pool(name="outp", bufs=1) as outp,
            tc.tile_pool(name="opsum", bufs=1, space="PSUM") as opsump,
            tc.tile_pool(name="tpsum", bufs=2, space="PSUM") as tpsump,
        ):
            ident = constp.tile([128, 128], dt.float16)
            make_identity(nc, ident[:])

            woff = constp.tile([128, K2 * 18], dt.float16)
            wconv = constp.tile([128, K2 * 128], dt.float16)
            xcm = xcmp.tile([128, 130 * 66], dt.float16)

            # prefetch superchunk 0 x-planes first, then the small stuff
            xq_tiles = []
            t0 = xqp.tile([128, SC_ELEMS], dt.float16, tag="xq")
            nc.sync.dma_start(out=t0[:], in_=xq_d[:, 0:SC_ELEMS])
            xq_tiles.append(t0)
            nc.sync.dma_start(out=xcm[:], in_=xcm_d[:])
            nc.sync.dma_start(out=woff[:], in_=woff_d[:])
            nc.sync.dma_start(out=wconv[:], in_=wconv_d[:])

            xcm3 = xcm[:].rearrange("p (h w) -> p h w", h=130, w=66)
            woff3 = woff[:].rearrange("p (k t) -> p k t", k=K2, t=18)
            wconv3 = wconv[:].rearrange("p (k o) -> p k o", k=K2, o=128)

            # ---- 1+2) offset conv + tri coefficients, per 16-w group so the
            # first superchunk's modulate can start after 1/4 of the conv ----
            # |d - m| written with bias in {0, 1} only (the only registered
            # const APs): |d+1| = Abs(d+1), |d| = Abs(d), |d-1| = Abs(-d+1)
            abs_args = [(1.0, 1.0), (1.0, 0.0), (-1.0, 1.0)]
            coefs = []
            for wg in range(NSC):
                wb = wg * SC_W
                offs = offp.tile([128, SC_W * 18], dt.float16, tag="offs")
                # per-w matmuls straight into the [h, (w, 18)] layout
                cpo = opsump.tile([128, SC_W * 32], dt.float32, tag="convpo")
                cpo3 = cpo[:].rearrange("p (w t) -> p w t", w=SC_W, t=32)
                for w in range(SC_W):
                    for k in range(K2):
                        ky, kx = k // 3, k % 3
                        lhsT = xcm3[:, ky : ky + 128, wb + w + kx]
                        nc.tensor.matmul(
                            cpo3[:, w, 0:18], lhsT, woff3[:, k, :],
                            start=(k == 0), stop=(k == K2 - 1),
                        )
                nc.scalar.copy(
                    offs[:].rearrange("p (w t) -> p w t", w=SC_W, t=18),
                    cpo3[:, :, 0:18],
                )
                # tri: [h, (m, k, w16)]
                offs_t = offs[:].rearrange("p (w t) -> p t w", w=SC_W, t=18)
                triy = coefp.tile([128, 3 * K2 * SC_W], dt.float16, tag="triy")
                trix = coefp.tile([128, 3 * K2 * SC_W], dt.float16, tag="trix")
                triy3 = triy[:].rearrange("p (m k w) -> p m k w", m=3, k=K2, w=SC_W)
                trix3 = trix[:].rearrange("p (m k w) -> p m k w", m=3, k=K2, w=SC_W)
                tmp = coefp.tile([128, K2 * SC_W], dt.float16, tag="tritmp")
                tmp2 = tmp[:].rearrange("p (k w) -> p k w", k=K2, w=SC_W)
                for ax in range(2):
                    src = offs_t[:, ax : 18 : 2, :]      # [p, k, w]
                    dstr = (triy3, trix3)[ax]
                    for mi, (sc_, bi_) in enumerate(abs_args):
                        nc.scalar.activation(
                            tmp2[:, :, :], src,
                            mybir.ActivationFunctionType.Abs,
                            bias=bi_, scale=sc_,
                        )
                        nc.scalar.activation(
                            dstr[:, mi, :, :], tmp2[:, :, :],
                            mybir.ActivationFunctionType.Relu,
                            scale=-1.0, bias=1.0,
                        )
                # coef[h, k, my, nx, w16] = triy[my,k,w] * trix[nx,k,w]
                coef = coefp.tile([128, K2 * 9 * SC_W], dt.float16, tag=f"coef{wg}")
                coef5 = coef[:].rearrange(
                    "p (k my nx w) -> p k my nx w", k=K2, my=3, nx=3, w=SC_W
                )
                for mi in range(3):
                    for ni in range(3):
                        nc.vector.tensor_tensor(
                            out=coef5[:, :, mi, ni, :],
                            in0=triy3[:, mi, :, :],
                            in1=trix3[:, ni, :, :],
                            op=mybir.AluOpType.mult,
                        )
                coefs.append(coef5)

            # ---- 3) per superchunk: modulate, transpose, contract ----
            for sc in range(NSC):
                if sc + 1 < NSC:
                    tnext = xqp.tile([128, SC_ELEMS], dt.float16, tag="xq")
                    nc.sync.dma_start(
                        out=tnext[:],
                        in_=xq_d[:, (sc + 1) * SC_ELEMS : (sc + 2) * SC_ELEMS],
                    )
                    xq_tiles.append(tnext)
                xqt = xq_tiles[sc]
                xq5 = xqt[:].rearrange(
                    "p (s q c w) -> p s q c w", s=NS, q=NQ, c=C, w=WQ
                )

                pout = opsump.tile([128, SC_W * 128], dt.float32, tag="po")

                def finish_unit(pend):
                    # deferred final adds (DMA-accumulated p2/p1 arrive with a
                    # full unit of slack), then transpose + contraction
                    samp3, p1, p2, kk = pend
                    for pr in (p2, p1):
                        if pr is not None:
                            nc.vector.tensor_tensor(
                                out=samp3[:, :, :], in0=samp3[:, :, :],
                                in1=pr[:, 0], op=mybir.AluOpType.add,
                            )
                    stage = stagep.tile([128, SC_W * 128], dt.float16, tag="stage")
                    stage3 = stage[:].rearrange("p (w h) -> p w h", w=SC_W, h=128)
                    for wi in range(SC_W):
                        tp = tpsump.tile([128, 128], dt.float16, tag="tp")
                        nc.tensor.transpose(tp[:], samp3[:, :, wi], ident[:])
                        nc.scalar.copy(stage3[:, wi, :], tp[:])
                    # matmul output must stay within one 512-f32 PSUM bank
                    for jh in range(4):
                        nc.tensor.matmul(
                            pout[:, jh * 512 : (jh + 1) * 512],
                            wconv3[:, kk, :],
                            stage[:, jh * 512 : (jh + 1) * 512],
                            start=(kk == 0),
                            stop=(kk == K2 - 1),
                        )

                pending = None
                for k in range(K2):
                    ky, kx = k // 3, k % 3

                    def mult(ni, eng):
                        sx = kx + ni - 2          # column shift -2..2
                        q = sx & 1
                        wa = sx + 2 - q           # even in-plane w offset
                        prod = prodp.tile(
                            [128, 3 * C * SC_W], dt.float16, tag="prod"
                        )
                        prod4 = prod[:].rearrange(
                            "p (m c w) -> p m c w", m=3, c=C, w=SC_W
                        )
                        cin = coefs[sc][:, k, :, ni, None, :].to_broadcast(
                            [128, 3, C, SC_W]
                        )
                        xin = xq5[:, ky : ky + 3, q, :, wa : wa + SC_W]
                        eng.tensor_tensor(
                            out=prod4[:, :, :, :], in0=cin, in1=xin,
                            op=mybir.AluOpType.mult,
                        )
                        return prod4

                    # ni=2 products first; both side-column reductions run as
                    # SDMA CCE accumulate-DMA chains (no DVE port contention);
                    # their consuming adds are deferred one unit so the DMA
                    # latency hides. Emission order keeps the 4-slot prod pool
                    # deadlock-free: the deferred adds free the previous
                    # unit's p1/p2 slots before m1/m0 reuse them.
                    p2 = mult(2, nc.vector)
                    if USE_DMA_ACCUM:
                        nc.gpsimd.dma_start(
                            out=p2[:, 0], in_=p2[:, 1],
                            accum_op=mybir.AluOpType.add,
                        )
                        nc.gpsimd.dma_start(
                            out=p2[:, 0], in_=p2[:, 2],
                            accum_op=mybir.AluOpType.add,
                        )
                        if pending is not None:
                            finish_unit(pending)
                    p1 = mult(1, nc.vector)
                    if USE_DMA_ACCUM:
                        nc.gpsimd.dma_start(
                            out=p1[:, 0], in_=p1[:, 1],
                            accum_op=mybir.AluOpType.add,
                        )
                    p0 = mult(0, nc.vector)

                    samp = sampp.tile([128, C * SC_W], dt.float16, tag="samp")
                    samp3 = samp[:].rearrange("p (c w) -> p c w", c=C, w=SC_W)
                    nc.vector.tensor_tensor(
                        out=samp3[:, :, :], in0=p0[:, 0], in1=p0[:, 1],
                        op=mybir.AluOpType.add,
                    )
                    tail = ((p0[:, 2], p1[:, 2], p1[:, 0])
                            if USE_DMA_ACCUM else
                            (p0[:, 2], p1[:, 0], p1[:, 1], p1[:, 2],
                             p2[:, 0], p2[:, 1], p2[:, 2]))
                    for src in tail:
                        nc.vector.tensor_tensor(
                            out=samp3[:, :, :], in0=samp3[:, :, :], in1=src,
                            op=mybir.AluOpType.add,
                        )
                    if USE_DMA_ACCUM:
                        pending = (samp3, None, p2, k)
                    else:
                        finish_unit((samp3, None, None, k))
                        pending = None
                if pending is not None:
                    finish_unit(pending)
                    pending = None

                osb = outp.tile([128, SC_W * 128], dt.float16, tag="osb")
                nc.scalar.copy(osb[:], pout[:])
                nc.sync.dma_start(
                    out=out_d[:, sc * SC_W * 128 : (sc + 1) * SC_W * 128],
                    in_=osb[:],
                )

    nc.finalize()
    return nc


def _host_shards(x, w_off, w_conv):
    """Prepare per-core input dicts."""
    woff_h = np.zeros((128, K2 * 18), dtype=f16)
    for k in range(K2):
        ky, kx = k // 3, k % 3
        woff_h[:, k * 18 : (k + 1) * 18] = w_off[:, :, ky, kx].T.astype(f16)
    wconv_h = np.zeros((128, K2 * 128), dtype=f16)
    for k in range(K2):
        ky, kx = k // 3, k % 3
        wconv_h[:, k * 128 : (k + 1) * 128] = w_conv[:, :, ky, kx].T.astype(f16)

    xb = x.astype(f16)
    ins = []
    for b in range(B):
        # padded image [c, h(-4..132), w(-4..132)] once per image
        xp = np.zeros((C, H + 8, W + 8), dtype=f16)
        xp[:, 4 : 4 + H, 4 : 4 + W] = xb[b]
        for s in range(2):
            ws = s * SW
            xcm = np.zeros((128, 130, 66), dtype=f16)
            c0, c1 = max(0, ws - 1), min(W, ws + 65)
            xcm[:, 1:129, (c0 - (ws - 1)) : (c1 - (ws - 1))] = xb[b][:, :, c0:c1]
            # xq[h, sc, si, q, c, w'] = x[c, h+si-2, ws+16*sc-2+q+w']
            xq = np.empty((128, NSC, NS, NQ, C, WQ), dtype=f16)
            for sc in range(NSC):
                base = ws + 16 * sc - 2
                for si in range(NS):
                    blk = xp[:, 4 + si - 2 : 4 + si - 2 + 128, :]  # [c, 128h, w+8]
                    for q in range(NQ):
                        wlo = 4 + base + q
                        xq[:, sc, si, q, :, :] = blk[:, :, wlo : wlo + WQ].transpose(1, 0, 2)
            ins.append(
                {
                    "xq": xq.reshape(128, -1),
                    "xcm": xcm.reshape(128, -1),
                    "woff": woff_h,
                    "wconv": wconv_h,
                }
            )
    return ins


_NC_CACHE = {}


def kernel(x, w_off, w_conv):
    from concourse.bass_utils import run_bass_kernel_spmd

    if "nc" not in _NC_CACHE:
        _NC_CACHE["nc"] = _build_nc()
    nc = _NC_CACHE["nc"]

    in_maps = _host_shards(np.asarray(x), np.asarray(w_off), np.asarray(w_conv))
    res = run_bass_kernel_spmd(nc, in_maps, core_ids=list(range(NCORES)))
    out = np.zeros((B, O, H, W), dtype=np.float32)
    for ci in range(NCORES):
        b, s = ci // 2, ci % 2
        ws = s * SW
        o = np.asarray(res.results[ci]["out"]).astype(np.float32)
        out[b, :, :, ws : ws + SW] = o.reshape(O, SW, H).transpose(0, 2, 1)
    return out
